# revision 1
# baseline (speedup 1.0000x reference)
"""Trainium2 Bass kernel for nn_CLModel_7370163880741 (gnn_message_passing).

Model: 64 independent conversation graphs (64 nodes each, banded +-8 window
adjacency), 2x RGCN layer -> TransformerConv (2 heads, local attention) ->
BatchNorm over all 4096 nodes -> per-node MLP head concatenated with the
sample's last node -> sigmoid.

Strategy (data-parallel over conversations, 8 samples / 512 nodes per core):
  * Graph structure is block-diagonal per sample -> the segment_sum message
    passing is a dense [128,128] matmul with a host-built per-chunk (2-sample)
    count/degree matrix; attention is dense masked softmax over 128-node
    chunks with a host-built log-count/-1e30 mask.
  * Layer 0's input is categorical (8 embeddings), so layer 0 collapses to
    rank-16: x0 = relu(U @ T0) with U = [inv_deg * (cnt @ onehot), onehot]
    (host) and T0 = [emb @ W_rel0; emb @ W_root0 + b_c0] (host).
  * bskip and bv shift every node's channel mean equally, so BatchNorm's
    mean subtraction cancels them exactly -> dropped.
  * Cross-core traffic: two [128,8] AllReduces of BN sum/sumsq (one per
    attention head, so the first overlaps with the second head's compute).
  * Compute dtype bf16 on the PE (fp32 PSUM accumulation), fp32 softmax/BN.

kernel(**inputs) takes FULL unsharded inputs (as produced by
setup_inputs()), shards by sample internally, runs the 8-core SPMD NEFF via
bass_utils.run_bass_kernel_spmd, and reassembles the full output.
"""

import math

import numpy as np
import ml_dtypes

import concourse.bass as bass
import concourse.mybir as mybir
import concourse.tile as tile
from concourse import bacc
from concourse.bass_utils import run_bass_kernel_spmd

# ---------------------------------------------------------------- constants
NCORES = 8
B, S = 64, 64
DIM, HEADS = 1024, 2
DH = DIM // 2                 # 512
NTOT = B * S                  # 4096
BL = B // NCORES              # 8 samples per core
NL = BL * S                   # 512 nodes per core
NCH = NL // 128               # 4 chunks (2 samples each)
KD = DIM // 128               # 8
KH = DH // 128                # 4
NEG = -1.0e30
EPS_BN = 1e-5

F32 = mybir.dt.float32
ACT_DT = mybir.dt.bfloat16    # PE/storage dtype for weights+activations
ACT_NP = ml_dtypes.bfloat16

AF = mybir.ActivationFunctionType
ALU = mybir.AluOpType

_COMPILED = None              # build cache: shapes are static
LAST_EXEC_NS = None
LAST_RESULTS = None


# ------------------------------------------------------------- host prep
def _host_prep(inputs):
    ii = {k: np.asarray(v) for k, v in inputs.items()}
    emotions = ii['emotions'].astype(np.int64).reshape(B, S)
    src = ii['src'].astype(np.int64)
    dst = ii['dst'].astype(np.int64)

    def f32(k):
        return np.asarray(ii[k], dtype=np.float32)

    sb, db = src // S, dst // S
    if not (sb == db).all():
        raise ValueError("edge list is not block-diagonal by sample")
    cnt = np.zeros((B, S, S), np.float32)
    np.add.at(cnt, (db, dst % S, src % S), 1.0)     # cnt[b, dst, src]
    deg = cnt.sum(axis=2)
    invdeg = (1.0 / np.maximum(deg, 1.0)).astype(np.float32)

    onehot = np.zeros((B, S, 8), np.float32)
    onehot[np.arange(B)[:, None], np.arange(S)[None, :], emotions] = 1.0

    U = np.zeros((B, S, 16), np.float32)
    U[..., :8] = invdeg[..., None] * np.einsum('bij,bje->bie', cnt, onehot)
    U[..., 8:] = onehot

    emb = f32('emb')
    T0 = np.concatenate(
        [emb @ f32('W_rel0'),
         emb @ f32('W_root0') + f32('b_c0')[None, :]], axis=0)  # [16, DIM]

    g2 = np.zeros((NCORES, 128, NCH, 128), np.float32)          # [p=j, n, i]
    amask = np.full((NCORES, 128, NCH, 128), NEG, np.float32)   # [p=i, n, j]
    for c in range(NCORES):
        for n in range(NCH):
            for s2 in range(2):
                b = c * BL + 2 * n + s2
                o = 64 * s2
                cb = cnt[b]
                g2[c, o:o + 64, n, o:o + 64] = (cb * invdeg[b][:, None]).T
                with np.errstate(divide='ignore'):
                    m = np.where(cb > 0, np.log(np.maximum(cb, 1e-30)), NEG)
                amask[c, o:o + 64, n, o:o + 64] = m

    sel = np.zeros((8, NL), np.float32)
    sel[np.arange(NL) // 64, np.arange(NL)] = 1.0

    def kpm(w, kchunks, m):                     # [K, M] -> [128, kchunks, m]
        return np.ascontiguousarray(
            w.reshape(kchunks, 128, m).transpose(1, 0, 2))

    W1 = f32('W1')
    scale_q = 1.0 / math.sqrt(DH)

    # fused attention score operator: scores = x^T M_h x + 1 (x) (w_h^T x).
    # (q = Wq^T x + bq, k = Wk^T x + bk; the per-row terms bq.k_j... wait
    #  bq.(k_j) varies over j and (q_i).bk + bq.bk are per-row/constant ->
    #  cancel under softmax-over-j; only (bq^T Wk^T x_j) survives.)
    Wqs, Wk_, bqs = f32('Wq') * scale_q, f32('Wk'), f32('bq') * scale_q
    Mh, wxh = [], []
    for h in range(HEADS):
        hsl = slice(h * DH, (h + 1) * DH)
        Mh.append(Wqs[:, hsl] @ Wk_[:, hsl].T)        # [DH, DH]
        wxh.append(Wk_[:, hsl] @ bqs[hsl])            # [DH]
    wxw = np.stack([w.reshape(KH, 128) for w in wxh], -1)  # [KH,128,2]
    wxw = np.ascontiguousarray(wxw.transpose(1, 0, 2))     # [128, KH, 2]

    consts = {
        't0': T0,                                              # [16, DIM]
        'wrel1': kpm(f32('W_rel1'), KD, DH),                   # [128, 8, 512]
        'wroot1': kpm(f32('W_root1'), KD, DH),
        'wv': kpm(f32('Wv'), KH, DIM),
        'wskip': kpm(f32('Wskip'), KH, DIM),
        'w1a': kpm(W1[:DIM], KD, DIM),                         # [128, 8, 1024]
        'w1b': kpm(W1[DIM:], KD, DIM),
        'w2': np.ascontiguousarray(f32('W2').reshape(KD, 128).T),  # [128, 8]
        'ma': kpm(Mh[0], KH, DH),                              # [128, 4, 512]
        'mb': kpm(Mh[1], KH, DH),
        'wxw': wxw,                                            # [128, 4, 2]
        'sel': sel,                                            # [8, NL]
        'ident': np.eye(128, dtype=np.float32),
        'ones_row': np.ones((1, NL), np.float32),
        'bc1': f32('b_c1').reshape(1, DH),
        'b1r': f32('b1').reshape(1, DIM),
        'gamma_t': np.ascontiguousarray(f32('bn_gamma').reshape(KD, 128).T),
        'beta_t': np.ascontiguousarray(f32('bn_beta').reshape(KD, 128).T),
        'negb2': np.array([[-float(np.asarray(ii['b2']).reshape(-1)[0])]],
                          np.float32),
    }

    per_core = []
    for c in range(NCORES):
        m = dict(consts)
        m['ut'] = np.ascontiguousarray(
            U[c * BL:(c + 1) * BL].reshape(NL, 16).T)          # [16, NL]
        m['g2'] = g2[c]
        m['amask'] = amask[c]
        per_core.append(m)
    return per_core


# dtype per dram input: bf16 for PE-facing tensors, f32 for stats-side ones
_F32_INPUTS = {'gamma_t', 'beta_t', 'negb2'}

_INPUT_SHAPES = {
    't0': (16, DIM), 'ut': (16, NL),
    'wrel1': (128, KD, DH), 'wroot1': (128, KD, DH),
    'wv': (128, KH, DIM), 'wskip': (128, KH, DIM),
    'ma': (128, KH, DH), 'mb': (128, KH, DH), 'wxw': (128, KH, 2),
    'w1a': (128, KD, DIM), 'w1b': (128, KD, DIM),
    'w2': (128, KD), 'sel': (8, NL), 'ident': (128, 128),
    'ones_row': (1, NL), 'bc1': (1, DH), 'b1r': (1, DIM),
    'gamma_t': (128, KD), 'beta_t': (128, KD), 'negb2': (1, 1),
    'g2': (128, NCH, 128), 'amask': (128, NCH, 128),
}

# first-use order for constant DMA issue (big late-use weights last)
_DMA_ORDER = [
    't0', 'ut', 'wrel1', 'g2', 'wroot1', 'bc1', 'ones_row',
    'ma', 'mb', 'wxw', 'wskip', 'wv', 'ident', 'amask',
    'gamma_t', 'beta_t',
    'w1b', 'w1a', 'sel', 'b1r', 'w2', 'negb2',
]


def _build_program():
    nc = bacc.Bacc("TRN2", target_bir_lowering=False, debug=False,
                   enable_asserts=False, num_devices=NCORES)

    dram = {}
    for name, shape in _INPUT_SHAPES.items():
        dt = F32 if name in _F32_INPUTS else ACT_DT
        dram[name] = nc.dram_tensor(name, list(shape), dt,
                                    kind="ExternalInput")
    s_out = nc.dram_tensor("s_out", [1, NL], F32, kind="ExternalOutput")

    with tile.TileContext(nc) as tc:
        _emit(nc, tc, dram, s_out)
    nc.compile()
    return nc


def _emit(nc, tc, dram, s_out):
    import contextlib
    ctx = contextlib.ExitStack()
    with ctx:
        consts = ctx.enter_context(tc.tile_pool(name="consts", bufs=1))
        acts = ctx.enter_context(tc.tile_pool(name="acts", bufs=1))
        tmp = ctx.enter_context(tc.tile_pool(name="tmp", bufs=3))
        pmm = ctx.enter_context(
            tc.tile_pool(name="pmm", bufs=2, space="PSUM"))
        dpool = ctx.enter_context(
            tc.tile_pool(name="dram", bufs=1, space="DRAM"))

        # single ACT table set load (Exp+Ln+Relu+Identity+Square all live in
        # natural_log_exp_and_others; without this walrus emits 3 loads)
        from concourse.hw_specs import get_activation_tables
        set_id = list(get_activation_tables(nc.m.arch)).index(
            'natural_log_exp_and_others')
        nc.scalar.add_instruction(mybir.InstLoadActFuncSet(
            name=nc.get_next_instruction_name(),
            act_func_set_id=set_id, ins=[], outs=[]))

        # ---- load constants into SBUF in first-use order
        cb = {}
        for name in _DMA_ORDER:
            shape = _INPUT_SHAPES[name]
            dt = F32 if name in _F32_INPUTS else ACT_DT
            t = consts.tile(list(shape), dt, name=f"c_{name}", tag=f"c_{name}")
            nc.sync.dma_start(out=t, in_=dram[name][:])
            cb[name] = t

        # fire a tiny unconsumed AllReduce immediately: wakes the ncfw/CC
        # path (first-collective ALGO_MESH wakeup measured ~11us) and lines
        # the cores up while input DMAs stream, so the BN collectives later
        # run warm.
        warm_src = acts.tile([1, 8], F32, tag="warm_src", name="warm_src")
        nc.vector.memset(warm_src, 0.0)
        warm_in = dpool.tile([1, 8], F32, name="warm_in")
        warm_out = dpool.tile([1, 8], F32, name="warm_out",
                              addr_space="Shared")
        nc.sync.dma_start(out=warm_in, in_=warm_src)
        nc.gpsimd.collective_compute(
            "AllReduce", ALU.add,
            replica_groups=[list(range(NCORES))],
            ins=[warm_in.opt()], outs=[warm_out.opt()])

        def mm(out, lhsT, rhs, start, stop=False):
            nc.tensor.matmul(out, lhsT, rhs, start=start, stop=stop,
                             skip_group_check=True)

        def move(idx, out, in_, relu=False, bias=None, scale=None):
            """PSUM->SBUF move alternating ACT/DVE to balance engines."""
            if idx % 2 == 0:
                func = AF.Relu if relu else AF.Identity
                kw = {}
                if bias is not None:
                    kw['bias'] = bias
                if scale is not None:
                    kw['scale'] = scale
                nc.scalar.activation(out=out, in_=in_, func=func, **kw)
            else:
                if relu:
                    nc.vector.tensor_scalar(out=out, in0=in_, scalar1=0.0,
                                            scalar2=None, op0=ALU.max)
                elif bias is not None and scale is not None:
                    nc.vector.tensor_scalar(out=out, in0=in_, scalar1=scale,
                                            scalar2=bias, op0=ALU.mult,
                                            op1=ALU.add)
                elif bias is not None:
                    nc.vector.tensor_scalar(out=out, in0=in_, scalar1=bias,
                                            scalar2=None, op0=ALU.add)
                else:
                    nc.vector.tensor_copy(out, in_)

        # ================= phase B: x0T[mc] = relu(T0^T @ U^T) ch-major
        x0T = []
        for mc in range(KD):
            ps = pmm.tile([128, NL], F32, tag="mm", name=f"ps_x0_{mc}")
            mm(ps, cb['t0'][:, mc * 128:(mc + 1) * 128], cb['ut'], True, True)
            t = acts.tile([128, NL], ACT_DT, tag=f"x0T{mc}", name=f"x0T{mc}")
            move(mc, t, ps, relu=True)
            x0T.append(t)

        # ================= phase C: msg[n] = (x0 @ Wrel1) node-major
        msg = []
        for n in range(NCH):
            ps = pmm.tile([128, DH], F32, tag="mm", name=f"ps_msg_{n}")
            for kc in range(KD):
                mm(ps, x0T[kc][:, n * 128:(n + 1) * 128],
                   cb['wrel1'][:, kc, :], kc == 0, kc == KD - 1)
            t = acts.tile([128, DH], ACT_DT, tag=f"msg{n}", name=f"msg{n}")
            move(n, t, ps)
            msg.append(t)

        # ================= phase D: x1T[cc] = relu(Wroot1^T x0 + bc1 + agg^T)
        x1T = []
        for cc in range(KH):
            ps = pmm.tile([128, NL], F32, tag="mm", name=f"ps_x1_{cc}")
            csl = slice(cc * 128, (cc + 1) * 128)
            for kc in range(KD):
                mm(ps, cb['wroot1'][:, kc, csl], x0T[kc], kc == 0)
            mm(ps, cb['bc1'][:, csl], cb['ones_row'], False)
            for n in range(NCH):
                mm(ps[:, n * 128:(n + 1) * 128], msg[n][:, csl],
                   cb['g2'][:, n, :], False, n == NCH - 1)
            t = acts.tile([128, NL], ACT_DT, tag=f"x1T{cc}", name=f"x1T{cc}")
            move(cc, t, ps, relu=True)
            x1T.append(t)

        # ================= phase E: fused score operator Y_h = M_h^T x1
        # (scores = Y_h^T x1 + mask + ones (x) wx_h) and v node-major
        Y = [[], []]
        wxsb = []
        for h in range(HEADS):
            wname = 'ma' if h == 0 else 'mb'
            for mc in range(KH):
                ps = pmm.tile([128, NL], F32, tag="mm",
                              name=f"ps_y{h}_{mc}")
                msl = slice(mc * 128, (mc + 1) * 128)
                for kc in range(KH):
                    mm(ps, cb[wname][:, kc, msl], x1T[kc], kc == 0,
                       kc == KH - 1)
                t = acts.tile([128, NL], ACT_DT, tag=f"y{h}_{mc}",
                              name=f"y{h}_{mc}")
                move(mc + h, t, ps)
                Y[h].append(t)
            ps = pmm.tile([1, NL], F32, tag="mm", name=f"ps_wx{h}")
            for kc in range(KH):
                mm(ps, cb['wxw'][:, kc, h:h + 1], x1T[kc], kc == 0,
                   kc == KH - 1)
            t = acts.tile([1, NL], ACT_DT, tag=f"wx{h}", name=f"wx{h}")
            nc.vector.tensor_copy(t, ps)
            wxsb.append(t)

        vsb = []
        for n in range(NCH):
            t = acts.tile([128, DIM], ACT_DT, tag=f"v{n}", name=f"v{n}")
            nsl = slice(n * 128, (n + 1) * 128)
            for half in range(2):
                ps = pmm.tile([128, DH], F32, tag="mm",
                              name=f"ps_v_{n}_{half}")
                hsl = slice(half * DH, (half + 1) * DH)
                for kc in range(KH):
                    mm(ps, x1T[kc][:, nsl], cb['wv'][:, kc, hsl], kc == 0,
                       kc == KH - 1)
                move(n + half, t[:, hsl], ps)
            vsb.append(t)

        # ================= phase F: per head skip+attention, split BN stats
        outpre = [None] * KD
        gstats = []        # per-head global stats tiles (after AllReduce)
        with tc.tile_pool(name="pskip", bufs=4, space="PSUM") as pskip, \
             tc.tile_pool(name="patt", bufs=2, space="PSUM") as patt:
            for h in range(HEADS):
                psk = []
                for mc4 in range(4):
                    m = h * 4 + mc4
                    ps = pskip.tile([128, NL], F32, tag="skip",
                                    name=f"ps_skip_{m}")
                    msl = slice(m * 128, (m + 1) * 128)
                    for kc in range(KH):
                        mm(ps, cb['wskip'][:, kc, msl], x1T[kc], kc == 0)
                    psk.append(ps)
                for n in range(NCH):
                    nsl = slice(n * 128, (n + 1) * 128)
                    ps = patt.tile([128, 128], F32, tag="att",
                                   name=f"ps_sc_{h}_{n}")
                    for kc4 in range(KH):
                        mm(ps, Y[h][kc4][:, nsl], x1T[kc4][:, nsl],
                           kc4 == 0)
                    mm(ps, cb['ident'], cb['amask'][:, n, :], False)
                    mm(ps, cb['ones_row'][:, :128], wxsb[h][:, nsl],
                       False, True)
                    negmax = tmp.tile([128, 1], F32, tag="negmax",
                                      name="negmax")
                    nc.vector.tensor_reduce(out=negmax, in_=ps,
                                            axis=mybir.AxisListType.X,
                                            op=ALU.max, negate=True)
                    probs = tmp.tile([128, 128], F32, tag="probs",
                                     name="probs")
                    denom = tmp.tile([128, 1], F32, tag="denom", name="denom")
                    nc.scalar.activation(out=probs, in_=ps, func=AF.Exp,
                                         bias=negmax, accum_out=denom)
                    rden = tmp.tile([128, 1], F32, tag="rden", name="rden")
                    nc.vector.reciprocal_approx_fast(out=rden, in_=denom)
                    alpha = tmp.tile([128, 128], ACT_DT, tag="alpha",
                                     name="alpha")
                    nc.vector.tensor_scalar(out=alpha, in0=probs,
                                            scalar1=rden, scalar2=None,
                                            op0=ALU.mult)
                    pt = patt.tile([128, 128], ACT_DT, tag="att",
                                   name=f"ps_at_{h}_{n}")
                    nc.tensor.transpose(pt, alpha, cb['ident'])
                    aT = tmp.tile([128, 128], ACT_DT, tag="aT", name="aT")
                    nc.vector.tensor_copy(aT, pt)
                    for mc4 in range(4):
                        m = h * 4 + mc4
                        mm(psk[mc4][:, nsl],
                           vsb[n][:, m * 128:(m + 1) * 128], aT, False,
                           n == NCH - 1)
                # stats fused into PSUM->SBUF moves via ACT accum_out
                # (cols 0..3 sums, 4..7 sum-of-squares for this head)
                stats = acts.tile([128, 8], F32, tag=f"stats{h}",
                                  name=f"stats{h}")
                for mc4 in range(4):
                    m = h * 4 + mc4
                    t = acts.tile([128, NL], F32, tag=f"outpre{m}",
                                  name=f"outpre{m}")
                    nc.scalar.activation(out=t, in_=psk[mc4],
                                         func=AF.Identity,
                                         accum_out=stats[:, mc4:mc4 + 1])
                    outpre[m] = t
                    sq = tmp.tile([128, NL], F32, tag="sqscratch",
                                  name="sqscratch")
                    nc.scalar.activation(out=sq, in_=psk[mc4],
                                         func=AF.Square,
                                         accum_out=stats[:, 4 + mc4:5 + mc4])
                # per-head BN stats AllReduce (head 0's overlaps head 1)
                bn_in = dpool.tile([128, 8], F32, name=f"bn_in{h}")
                bn_out = dpool.tile([128, 8], F32, name=f"bn_out{h}",
                                    addr_space="Shared")
                nc.sync.dma_start(out=bn_in, in_=stats)
                nc.gpsimd.collective_compute(
                    "AllReduce", ALU.add,
                    replica_groups=[list(range(NCORES))],
                    ins=[bn_in.opt()], outs=[bn_out.opt()])
                g = acts.tile([128, 8], F32, tag=f"gstats{h}",
                              name=f"gstats{h}")
                nc.sync.dma_start(out=g, in_=bn_out)
                gstats.append(g)

        # ================= phase G: per-head BN math + normalize
        epsc = acts.tile([128, 1], F32, tag="epsc", name="epsc")
        nc.vector.memset(epsc, EPS_BN)
        bn = [None] * KD
        for h in range(HEADS):
            g = gstats[h]
            hs = slice(h * 4, h * 4 + 4)
            mean = acts.tile([128, 4], F32, tag=f"bn_mean{h}",
                             name=f"bn_mean{h}")
            nc.vector.tensor_scalar(out=mean, in0=g[:, 0:4],
                                    scalar1=1.0 / NTOT, scalar2=None,
                                    op0=ALU.mult)
            var = acts.tile([128, 4], F32, tag=f"bn_var{h}",
                            name=f"bn_var{h}")
            nc.vector.tensor_scalar(out=var, in0=g[:, 4:8],
                                    scalar1=1.0 / NTOT, scalar2=None,
                                    op0=ALU.mult)
            msq = tmp.tile([128, 4], F32, tag="bn_msq", name="bn_msq")
            nc.vector.tensor_tensor(out=msq, in0=mean, in1=mean, op=ALU.mult)
            nc.vector.tensor_tensor(out=var, in0=var, in1=msq,
                                    op=ALU.subtract)
            # rstd = exp(-0.5*ln(var+eps)) stays inside the one ACT table set
            lnv = tmp.tile([128, 4], F32, tag="bn_lnv", name="bn_lnv")
            nc.scalar.activation(out=lnv, in_=var, func=AF.Ln, bias=epsc)
            rstd = tmp.tile([128, 4], F32, tag="bn_rstd", name="bn_rstd")
            nc.scalar.activation(out=rstd, in_=lnv, func=AF.Exp, scale=-0.5)
            sg = acts.tile([128, 4], F32, tag=f"bn_sg{h}", name=f"bn_sg{h}")
            nc.vector.tensor_tensor(out=sg, in0=cb['gamma_t'][:, hs],
                                    in1=rstd, op=ALU.mult)
            shift = acts.tile([128, 4], F32, tag=f"bn_shift{h}",
                              name=f"bn_shift{h}")
            nc.vector.tensor_tensor(out=shift, in0=mean, in1=sg, op=ALU.mult)
            nc.vector.tensor_tensor(out=shift, in0=cb['beta_t'][:, hs],
                                    in1=shift, op=ALU.subtract)
            for mc4 in range(4):
                m = h * 4 + mc4
                t = acts.tile([128, NL], ACT_DT, tag=f"bn{m}", name=f"bn{m}")
                move(m, t, outpre[m], bias=shift[:, mc4:mc4 + 1],
                     scale=sg[:, mc4:mc4 + 1])
                bn[m] = t

        # ================= phase H: MLP head (k-outer so W1 work on head-0
        # channels can start while head-1's collective is in flight; m split
        # in 2 passes of 4 PSUM banks)
        with tc.tile_pool(name="ph", bufs=6, space="PSUM") as phl:
            # tT[s, m] = tgtcols^T @ W1b : psum [8, 512] x2
            tT = acts.tile([8, DIM], ACT_DT, tag="tT", name="tT")
            for half in range(2):
                ps = pmm.tile([8, DH], F32, tag="mm", name=f"ps_tT_{half}")
                hsl = slice(half * DH, (half + 1) * DH)
                mm(ps, cb['ones_row'][:, :8], cb['b1r'][:, hsl], True)
                for kc in range(KD):
                    tgtcols = bn[kc].rearrange(
                        "p (s t) -> p s t", t=64)[:, :, 63:64]
                    mm(ps, tgtcols, cb['w1b'][:, kc, hsl], False,
                       kc == KD - 1)
                nc.vector.tensor_copy(tT[:, hsl], ps)

            hsb = [None] * KD
            for ms in ([0, 1, 2, 3, 4, 5], [6, 7]):
                ph = {m: phl.tile([128, NL], F32, tag="hpsum",
                                  name=f"ps_h_{m}") for m in ms}
                for kc in range(KD):
                    for m in ms:
                        msl = slice(m * 128, (m + 1) * 128)
                        mm(ph[m], cb['w1a'][:, kc, msl], bn[kc], kc == 0)
                for m in ms:
                    msl = slice(m * 128, (m + 1) * 128)
                    mm(ph[m], tT[:, msl], cb['sel'], False, True)
                    t = acts.tile([128, NL], ACT_DT, tag=f"h{m}",
                                  name=f"h{m}")
                    move(m, t, ph[m], relu=True)
                    hsb[m] = t

            pz = pmm.tile([1, NL], F32, tag="mm", name="ps_z")
            for m in range(KD):
                mm(pz, cb['w2'][:, m:m + 1], hsb[m], m == 0, m == KD - 1)
            esb = acts.tile([1, NL], F32, tag="esb", name="esb")
            nc.scalar.activation(out=esb, in_=pz, func=AF.Exp, scale=-1.0,
                                 bias=cb['negb2'][:, 0:1])
            nc.vector.tensor_scalar(out=esb, in0=esb, scalar1=1.0,
                                    scalar2=None, op0=ALU.add)
            ssb = acts.tile([1, NL], F32, tag="ssb", name="ssb")
            nc.vector.reciprocal_approx_fast(out=ssb, in_=esb)
            nc.sync.dma_start(out=s_out[:], in_=ssb)


# ------------------------------------------------------------------ driver
def kernel(_bass_trace=False, **inputs):
    global _COMPILED, LAST_EXEC_NS, LAST_RESULTS
    per_core = _host_prep(inputs)

    if _COMPILED is None:
        _COMPILED = _build_program()
    nc = _COMPILED

    in_maps = []
    for c in range(NCORES):
        m = {}
        for name in _INPUT_SHAPES:
            npdt = np.float32 if name in _F32_INPUTS else ACT_NP
            m[name] = np.ascontiguousarray(per_core[c][name], dtype=npdt)
        in_maps.append(m)

    res = run_bass_kernel_spmd(nc, in_maps, list(range(NCORES)),
                               trace=_bass_trace)
    LAST_EXEC_NS = res.exec_time_ns
    LAST_RESULTS = res

    f = np.full((B, 512), -1.0, np.float32)
    for c in range(NCORES):
        f[c * BL:(c + 1) * BL, :S] = \
            np.asarray(res.results[c]['s_out'], np.float32).reshape(BL, S)
    mask = np.zeros((B, 512), np.int32)
    mask[:, :S] = 1
    return f, mask



# revision 13
# speedup vs baseline: 1.0806x; 1.0806x over previous
"""Trainium2 Bass kernel for nn_CLModel_7370163880741 (gnn_message_passing).

Model: 64 independent conversation graphs (64 nodes each, banded +-8 window
adjacency), 2x RGCN layer -> TransformerConv (2 heads, local attention) ->
BatchNorm over all 4096 nodes -> per-node MLP head concatenated with the
sample's last node -> sigmoid.

Strategy (data-parallel over conversations, 8 samples / 512 nodes per core):
  * Graph structure is block-diagonal per sample -> the segment_sum message
    passing is a dense [128,128] matmul with a host-built per-chunk (2-sample)
    count/degree matrix; attention is dense masked softmax over 128-node
    chunks with a host-built log-count/-1e30 mask.
  * Layer 0's input is categorical (8 embeddings), so layer 0 collapses to
    rank-16: x0 = relu(U @ T0) with U = [inv_deg * (cnt @ onehot), onehot]
    (host) and T0 = [emb @ W_rel0; emb @ W_root0 + b_c0] (host).
  * bskip and bv shift every node's channel mean equally, so BatchNorm's
    mean subtraction cancels them exactly -> dropped. Other all-zero biases
    (checked at runtime) are elided at build time.
  * Attention-score path (M_h = Wq_h Wk_h^T / sqrt(dh), Y = M^T x1, and the
    Y^T x1 score matmuls) runs in fp8e4 DoubleRow (2x PE throughput); the
    softmax renormalization makes score quantization error negligible.
    Optionally (W1_FP8) the post-BN MLP GEMM also runs fp8 DoubleRow.
  * Softmax skips the running-max subtraction: true logits are |q.k/sqrt(dh)|
    <~ 0.1 plus a log-degree mask <= log(17), so exp() stays in [e-3, e+3].
  * Cross-core traffic: ONE [128,16] AllReduce of BN sum/sumsq for both
    heads; a tiny unconsumed warm-up AllReduce is issued before any weight
    DMA so the first-collective barrier/wakeup overlaps the forward pass.
  * Compute dtype bf16 on the PE (fp32 PSUM accumulation), fp32 softmax/BN.

kernel(**inputs) takes FULL unsharded inputs (as produced by
setup_inputs()), shards by sample internally, runs the 8-core SPMD NEFF via
bass_utils.run_bass_kernel_spmd, and reassembles the full output.
"""

import math
import os

import numpy as np
import ml_dtypes

_DEBUG_CUT = int(os.environ.get('KERNEL_DEBUG_CUT', '0'))

import concourse.bass as bass
import concourse.mybir as mybir
import concourse.tile as tile
from concourse import bacc
from concourse.bass_utils import run_bass_kernel_spmd

# ---------------------------------------------------------------- constants
NCORES = 8
B, S = 64, 64
DIM, HEADS = 1024, 2
DH = DIM // 2                 # 512
NTOT = B * S                  # 4096
BL = B // NCORES              # 8 samples per core
NL = BL * S                   # 512 nodes per core
NCH = NL // 128               # 4 chunks (2 samples each)
KD = DIM // 128               # 8
KH = DH // 128                # 4
NEG = -1.0e30
EPS_BN = 1e-5

F32 = mybir.dt.float32
BF16 = mybir.dt.bfloat16
FP8 = mybir.dt.float8e4
BF16_NP = ml_dtypes.bfloat16
FP8_NP = ml_dtypes.float8_e4m3fn

AF = mybir.ActivationFunctionType
ALU = mybir.AluOpType
DR = mybir.MatmulPerfMode.DoubleRow

W1_FP8 = True                 # post-BN MLP GEMM in fp8 DoubleRow

# fp8 scale plan (power-of-2 scales; unscaled in PSUM->SBUF moves):
S_X1 = 8.0                    # x1 fp8 shadow (score path only)
S_M = 2048.0                  # fused score operator M_h (host-scaled)
S_Y = 256.0                   # Y = M^T x1 fp8 tile
SC_SCORE = S_Y * S_X1         # score psum carries 2048 * true logits
S_BN = 4.0 if W1_FP8 else 1.0
S_W1 = 32.0                   # W1/W2 host fp8 scale
S_H = 8.0                     # hidden MLP activation fp8 scale
MLP_DT = FP8 if W1_FP8 else BF16

_COMPILED = {}                # build cache keyed by bias flags
LAST_EXEC_NS = None
LAST_RESULTS = None


def _q8(x, s):
    return np.clip(np.asarray(x, np.float32) * s, -240.0, 240.0)


# ------------------------------------------------------------- host prep
def _host_prep(inputs):
    ii = {k: np.asarray(v) for k, v in inputs.items()}
    emotions = ii['emotions'].astype(np.int64).reshape(B, S)
    src = ii['src'].astype(np.int64)
    dst = ii['dst'].astype(np.int64)

    def f32(k):
        return np.asarray(ii[k], dtype=np.float32)

    sb, db = src // S, dst // S
    if not (sb == db).all():
        raise ValueError("edge list is not block-diagonal by sample")
    cnt = np.zeros((B, S, S), np.float32)
    np.add.at(cnt, (db, dst % S, src % S), 1.0)     # cnt[b, dst, src]
    deg = cnt.sum(axis=2)
    invdeg = (1.0 / np.maximum(deg, 1.0)).astype(np.float32)

    onehot = np.zeros((B, S, 8), np.float32)
    onehot[np.arange(B)[:, None], np.arange(S)[None, :], emotions] = 1.0

    U = np.zeros((B, S, 16), np.float32)
    U[..., :8] = invdeg[..., None] * np.einsum('bij,bje->bie', cnt, onehot)
    U[..., 8:] = onehot

    emb = f32('emb')
    T0 = np.concatenate(
        [emb @ f32('W_rel0'),
         emb @ f32('W_root0') + f32('b_c0')[None, :]], axis=0)  # [16, DIM]

    g2 = np.zeros((NCORES, 128, NCH, 128), np.float32)          # [p=j, n, i]
    amask = np.full((NCORES, 128, NCH, 128), NEG, np.float32)   # [p=i, n, j]
    for c in range(NCORES):
        for n in range(NCH):
            for s2 in range(2):
                b = c * BL + 2 * n + s2
                o = 64 * s2
                cb = cnt[b]
                g2[c, o:o + 64, n, o:o + 64] = (cb * invdeg[b][:, None]).T
                with np.errstate(divide='ignore'):
                    m = np.where(cb > 0,
                                 np.log(np.maximum(cb, 1e-30)) * SC_SCORE,
                                 NEG)
                amask[c, o:o + 64, n, o:o + 64] = m

    sel = np.zeros((8, NL), np.float32)
    sel[np.arange(NL) // 64, np.arange(NL)] = 1.0

    def kpm(w, kchunks, m):                     # [K, M] -> [128, kchunks, m]
        return np.ascontiguousarray(
            w.reshape(kchunks, 128, m).transpose(1, 0, 2))

    W1 = f32('W1')
    scale_q = 1.0 / math.sqrt(DH)

    # fused attention score operator: scores = x^T M_h x (+ bq^T Wk^T x_j
    # when bq != 0; per-row terms cancel under softmax-over-j).
    Wqs, Wk_, bqs = f32('Wq') * scale_q, f32('Wk'), f32('bq') * scale_q
    Mh, wxh = [], []
    for h in range(HEADS):
        hsl = slice(h * DH, (h + 1) * DH)
        Mh.append(Wqs[:, hsl] @ Wk_[:, hsl].T)        # [DH, DH]
        wxh.append(Wk_[:, hsl] @ bqs[hsl])            # [DH]
    wxw = np.stack([w.reshape(KH, 128) for w in wxh], -1)  # [KH,128,2]
    wxw = np.ascontiguousarray(wxw.transpose(1, 0, 2)) * SC_SCORE / S_X1

    w1s = S_W1 if W1_FP8 else 1.0
    consts = {
        't0': T0,                                              # [16, DIM]
        'wrel1': kpm(f32('W_rel1'), KD, DH),                   # [128, 8, 512]
        'wroot1': kpm(f32('W_root1'), KD, DH),
        'wv': kpm(f32('Wv'), KH, DIM),
        'wskip': kpm(f32('Wskip'), KH, DIM),
        'w1a': _q8(kpm(W1[:DIM], KD, DIM), w1s),               # [128, 8, 1024]
        'w1b': _q8(kpm(W1[DIM:], KD, DIM), w1s),
        'w2': np.ascontiguousarray(f32('W2').reshape(KD, 128).T),
        'ma': _q8(kpm(Mh[0], KH, DH), S_M),                    # [128, 4, 512]
        'mb': _q8(kpm(Mh[1], KH, DH), S_M),
        'wxw': wxw,                                            # [128, 4, 2]
        'sel': sel,                                            # [8, NL]
        'ident': np.eye(128, dtype=np.float32),
        'ones_row': np.ones((1, NL), np.float32),
        'bc1': f32('b_c1').reshape(1, DH),
        'b1r': f32('b1').reshape(1, DIM) * (S_BN * w1s),
        'gamma_t': np.ascontiguousarray(
            f32('bn_gamma').reshape(KD, 128).T) * S_BN,
        'beta_t': np.ascontiguousarray(
            f32('bn_beta').reshape(KD, 128).T) * S_BN,
        'negb2': np.array([[-float(np.asarray(ii['b2']).reshape(-1)[0])]],
                          np.float32) * (1.0 if not W1_FP8 else S_H * S_W1),
    }

    flags = (bool(np.any(f32('b_c1'))), bool(np.any(f32('bq'))),
             bool(np.any(f32('b1'))), bool(np.any(f32('b2'))))

    per_core = []
    for c in range(NCORES):
        m = dict(consts)
        m['ut'] = np.ascontiguousarray(
            U[c * BL:(c + 1) * BL].reshape(NL, 16).T)          # [16, NL]
        m['g2'] = g2[c]
        m['amask'] = amask[c]
        per_core.append(m)
    return per_core, flags


# dtype per dram input
_F32_INPUTS = {'gamma_t', 'beta_t', 'negb2'}
_FP8_INPUTS = {'ma', 'mb'} | ({'w1a', 'w1b'} if W1_FP8 else set())

_INPUT_SHAPES = {
    't0': (16, DIM), 'ut': (16, NL),
    'wrel1': (128, KD, DH), 'wroot1': (128, KD, DH),
    'wv': (128, KH, DIM), 'wskip': (128, KH, DIM),
    'ma': (128, KH, DH), 'mb': (128, KH, DH), 'wxw': (128, KH, 2),
    'w1a': (128, KD, DIM), 'w1b': (128, KD, DIM),
    'w2': (128, KD), 'sel': (8, NL), 'ident': (128, 128),
    'ones_row': (1, NL), 'bc1': (1, DH), 'b1r': (1, DIM),
    'gamma_t': (128, KD), 'beta_t': (128, KD), 'negb2': (1, 1),
    'g2': (128, NCH, 128), 'amask': (128, NCH, 128),
}

# first-use order for constant DMA issue
_DMA_ORDER = [
    't0', 'ut', 'wrel1', 'g2', 'wroot1', 'bc1', 'ones_row',
    'ma', 'mb', 'wxw', 'wv', 'wskip', 'ident', 'amask',
    'gamma_t', 'beta_t',
    'w1a', 'w1b', 'sel', 'b1r', 'w2', 'negb2',
]


def _np_dt(name):
    if name in _F32_INPUTS:
        return np.float32
    if name in _FP8_INPUTS:
        return FP8_NP
    return BF16_NP


def _my_dt(name):
    if name in _F32_INPUTS:
        return F32
    if name in _FP8_INPUTS:
        return FP8
    return BF16


def _build_program(flags):
    nc = bacc.Bacc("TRN2", target_bir_lowering=False, debug=False,
                   enable_asserts=False, num_devices=NCORES)

    dram = {}
    for name, shape in _INPUT_SHAPES.items():
        dram[name] = nc.dram_tensor(name, list(shape), _my_dt(name),
                                    kind="ExternalInput")
    s_out = nc.dram_tensor("s_out", [1, NL], F32, kind="ExternalOutput")

    with tile.TileContext(nc) as tc:
        _emit(nc, tc, dram, s_out, flags)
    nc.compile()
    return nc


def _emit(nc, tc, dram, s_out, flags):
    has_bc1, has_bq, has_b1, has_b2 = flags
    import contextlib
    ctx = contextlib.ExitStack()
    with ctx:
        consts = ctx.enter_context(tc.tile_pool(name="consts", bufs=1))
        acts = ctx.enter_context(tc.tile_pool(name="acts", bufs=1))
        tmp = ctx.enter_context(tc.tile_pool(name="tmp", bufs=3))
        pmm = ctx.enter_context(
            tc.tile_pool(name="pmm", bufs=2, space="PSUM"))
        dpool = ctx.enter_context(
            tc.tile_pool(name="dram", bufs=1, space="DRAM"))

        # single ACT table set load (Exp+Ln+Relu+Identity+Square)
        from concourse.hw_specs import get_activation_tables
        set_id = list(get_activation_tables(nc.m.arch)).index(
            'natural_log_exp_and_others')
        nc.scalar.add_instruction(mybir.InstLoadActFuncSet(
            name=nc.get_next_instruction_name(),
            act_func_set_id=set_id, ins=[], outs=[]))

        # ---- load constants into SBUF in first-use order; the warm-up
        # collective goes right after the first tiny DMA (a collective as
        # the very first queue activity wedges the device), so its trigger
        # fires within ~1us and the first-collective barrier/ncfw wakeup
        # overlaps the whole forward pass instead of gating the BN stats
        # AllReduce.
        cb = {}
        for name in _DMA_ORDER:
            shape = _INPUT_SHAPES[name]
            t = consts.tile(list(shape), _my_dt(name), name=f"c_{name}",
                            tag=f"c_{name}")
            nc.sync.dma_start(out=t, in_=dram[name][:])
            cb[name] = t
            if name == 't0':
                warm_src = acts.tile([1, 8], F32, tag="warm_src",
                                     name="warm_src")
                nc.vector.memset(warm_src, 0.0)
                warm_in = dpool.tile([1, 8], F32, name="warm_in")
                warm_out = dpool.tile([1, 8], F32, name="warm_out",
                                      addr_space="Shared")
                nc.sync.dma_start(out=warm_in, in_=warm_src)
                nc.gpsimd.collective_compute(
                    "AllReduce", ALU.add,
                    replica_groups=[list(range(NCORES))],
                    ins=[warm_in.opt()], outs=[warm_out.opt()])

        def mm(out, lhsT, rhs, start, stop=False):
            nc.tensor.matmul(out, lhsT, rhs, start=start, stop=stop,
                             skip_group_check=True)

        def mmdr(out, lhsT, rhs, start, stop=False):
            nc.tensor.matmul(out, lhsT, rhs, start=start, stop=stop,
                             perf_mode=DR, skip_group_check=True)

        def move(idx, out, in_, relu=False, bias=None, scale=None,
                 accum_out=None):
            """PSUM->SBUF move alternating ACT/DVE to balance engines."""
            if idx % 2 == 0:
                func = AF.Relu if relu else AF.Identity
                kw = {}
                if bias is not None:
                    kw['bias'] = bias
                if scale is not None:
                    kw['scale'] = scale
                if accum_out is not None:
                    kw['accum_out'] = accum_out
                nc.scalar.activation(out=out, in_=in_, func=func, **kw)
            else:
                kw = {}
                if accum_out is not None:
                    kw['accum_out'] = accum_out
                if relu:
                    if scale is not None:
                        nc.vector.tensor_scalar(out=out, in0=in_,
                                                scalar1=scale, scalar2=0.0,
                                                op0=ALU.mult, op1=ALU.max,
                                                **kw)
                    else:
                        nc.vector.tensor_scalar(out=out, in0=in_,
                                                scalar1=0.0, scalar2=None,
                                                op0=ALU.max, **kw)
                elif bias is not None and scale is not None:
                    nc.vector.tensor_scalar(out=out, in0=in_, scalar1=scale,
                                            scalar2=bias, op0=ALU.mult,
                                            op1=ALU.add, **kw)
                elif scale is not None:
                    nc.vector.tensor_scalar(out=out, in0=in_, scalar1=scale,
                                            scalar2=None, op0=ALU.mult, **kw)
                else:
                    nc.vector.tensor_copy(out, in_, **kw)

        # ================= phase B: x0T[:, mc, :] = relu(T0^T @ U^T)
        x0T = acts.tile([128, KD, NL], BF16, tag="x0T", name="x0T")
        for mc in range(KD):
            ps = pmm.tile([128, NL], F32, tag="mm", name=f"ps_x0_{mc}")
            mm(ps, cb['t0'][:, mc * 128:(mc + 1) * 128], cb['ut'], True, True)
            move(mc, x0T[:, mc, :], ps, relu=True)

        # ================= phase C: msg[n] = (x0 @ Wrel1) node-major, bf16
        msg = []
        for n in range(NCH):
            ps = pmm.tile([128, DH], F32, tag="mm", name=f"ps_msg_{n}")
            nsl = slice(n * 128, (n + 1) * 128)
            for kc in range(KD):
                mm(ps, x0T[:, kc, nsl], cb['wrel1'][:, kc, :],
                   kc == 0, kc == KD - 1)
            t = acts.tile([128, DH], BF16, tag=f"msg{n}", name=f"msg{n}")
            move(n, t, ps)
            msg.append(t)

        # ================= phase D: x1T = relu(Wroot1^T x0 + agg^T) (+bc1)
        # bf16 primary + fp8 shadow (score path)
        x1T = acts.tile([128, KH, NL], BF16, tag="x1T", name="x1T")
        x1F = acts.tile([128, KH, NL], FP8, tag="x1F", name="x1F")
        for cc in range(KH):
            ps = pmm.tile([128, NL], F32, tag="mm", name=f"ps_x1_{cc}")
            csl = slice(cc * 128, (cc + 1) * 128)
            for kc in range(KD):
                mm(ps, cb['wroot1'][:, kc, csl], x0T[:, kc, :], kc == 0)
            if has_bc1:
                mm(ps, cb['bc1'][:, csl], cb['ones_row'], False)
            for n in range(NCH):
                mm(ps[:, n * 128:(n + 1) * 128], msg[n][:, csl],
                   cb['g2'][:, n, :], False, n == NCH - 1)
            nc.scalar.activation(out=x1T[:, cc, :], in_=ps, func=AF.Relu)
            nc.vector.tensor_scalar(out=x1F[:, cc, :], in0=ps,
                                    scalar1=S_X1, scalar2=0.0,
                                    op0=ALU.mult, op1=ALU.max)

        def _cut(src):
            t = acts.tile([1, NL], F32, tag="cutout", name="cutout")
            nc.vector.tensor_copy(t, src)
            nc.sync.dma_start(out=s_out[:], in_=t)

        if _DEBUG_CUT == 1:           # end after phase D
            _cut(x1T[0:1, 0, :])
            return

        # ================= phase E: Y_h = M_h^T x1 (fp8 DoubleRow) and v
        Y = []
        wxsb = []
        for h in range(HEADS):
            wname = 'ma' if h == 0 else 'mb'
            yt = acts.tile([128, KH, NL], FP8, tag=f"y{h}", name=f"y{h}")
            for mc in range(KH):
                ps = pmm.tile([128, NL], F32, tag="mm", name=f"ps_y{h}_{mc}")
                msl = slice(mc * 128, (mc + 1) * 128)
                for j in range(KH // 2):
                    jsl = slice(2 * j, 2 * j + 2)
                    mmdr(ps, cb[wname][:, jsl, msl], x1F[:, jsl, :],
                         j == 0, j == KH // 2 - 1)
                # psum = S_M*S_X1*Y ; store Y_fp8 = S_Y*Y
                move(mc + h, yt[:, mc, :], ps, scale=S_Y / (S_M * S_X1))
            Y.append(yt)
            if has_bq:
                ps = pmm.tile([1, NL], F32, tag="mm", name=f"ps_wx{h}")
                for kc in range(KH):
                    mm(ps, cb['wxw'][:, kc, h:h + 1], x1F[:, kc, :], kc == 0,
                       kc == KH - 1)
                t = acts.tile([1, NL], BF16, tag=f"wx{h}", name=f"wx{h}")
                nc.vector.tensor_copy(t, ps)
                wxsb.append(t)

        vsb = []
        for n in range(NCH):
            t = acts.tile([128, DIM], BF16, tag=f"v{n}", name=f"v{n}")
            nsl = slice(n * 128, (n + 1) * 128)
            for half in range(2):
                ps = pmm.tile([128, DH], F32, tag="mm",
                              name=f"ps_v_{n}_{half}")
                hsl = slice(half * DH, (half + 1) * DH)
                for kc in range(KH):
                    mm(ps, x1T[:, kc, nsl], cb['wv'][:, kc, hsl], kc == 0,
                       kc == KH - 1)
                move(n + half, t[:, hsl], ps)
            vsb.append(t)

        if _DEBUG_CUT == 2:           # end after phase E
            _cut(vsb[0][0:1, :NL])
            return

        # ================= phase F: per head skip+attention, fused BN stats
        outpre = acts.tile([128, KD, NL], BF16, tag="outpre", name="outpre")
        stats = acts.tile([128, 16], F32, tag="stats", name="stats")
        with tc.tile_pool(name="pskip", bufs=4, space="PSUM") as pskip, \
             tc.tile_pool(name="patt", bufs=2, space="PSUM") as patt:
            for h in range(HEADS):
                # scores for all chunks first (PE), softmax trails on ACT/DVE
                scps = patt.tile([128, NCH, 128], F32, tag="att",
                                 name=f"ps_sc_{h}")
                for n in range(NCH):
                    nsl = slice(n * 128, (n + 1) * 128)
                    for j in range(KH // 2):
                        jsl = slice(2 * j, 2 * j + 2)
                        mmdr(scps[:, n, :], Y[h][:, jsl, nsl],
                             x1F[:, jsl, nsl], j == 0, False)
                    if has_bq:
                        mm(scps[:, n, :], cb['ones_row'][:, :128],
                           wxsb[h][:, nsl], False)
                    mm(scps[:, n, :], cb['ident'], cb['amask'][:, n, :],
                       False, True)
                psk = []
                for mc4 in range(4):
                    m = h * 4 + mc4
                    ps = pskip.tile([128, NL], F32, tag="skip",
                                    name=f"ps_skip_{m}")
                    msl = slice(m * 128, (m + 1) * 128)
                    for kc in range(KH):
                        mm(ps, cb['wskip'][:, kc, msl], x1T[:, kc, :],
                           kc == 0)
                    psk.append(ps)
                ptp = patt.tile([128, NCH, 128], BF16, tag="att",
                                name=f"ps_at_{h}")
                for n in range(NCH):
                    nsl = slice(n * 128, (n + 1) * 128)
                    # no max-subtraction: scores bounded (see module doc)
                    probs = tmp.tile([128, 128], F32, tag="probs",
                                     name="probs")
                    denom = tmp.tile([128, 1], F32, tag="denom", name="denom")
                    nc.scalar.activation(out=probs, in_=scps[:, n, :],
                                         func=AF.Exp, scale=1.0 / SC_SCORE,
                                         accum_out=denom)
                    rden = tmp.tile([128, 1], F32, tag="rden", name="rden")
                    nc.vector.reciprocal_approx_fast(out=rden, in_=denom)
                    alpha = tmp.tile([128, 128], BF16, tag="alpha",
                                     name="alpha")
                    nc.vector.tensor_scalar(out=alpha, in0=probs,
                                            scalar1=rden, scalar2=None,
                                            op0=ALU.mult)
                    nc.tensor.transpose(ptp[:, n, :], alpha, cb['ident'])
                    aT = tmp.tile([128, 128], BF16, tag="aT", name="aT")
                    move(n, aT, ptp[:, n, :])
                    for mc4 in range(4):
                        m = h * 4 + mc4
                        mm(psk[mc4][:, nsl],
                           vsb[n][:, m * 128:(m + 1) * 128], aT, False,
                           n == NCH - 1)
                # stats fused into the PSUM->SBUF moves via ACT accum_out
                # (cols 0..7 sums, 8..15 sum-of-squares; TensorTensorReduce
                # on DVE wedges the device at runtime, so both passes ride
                # the scalar engine)
                for mc4 in range(4):
                    m = h * 4 + mc4
                    nc.scalar.activation(out=outpre[:, m, :], in_=psk[mc4],
                                         func=AF.Identity,
                                         accum_out=stats[:, m:m + 1])
                    sq = tmp.tile([128, NL], BF16, tag="sqscratch",
                                  name="sqscratch")
                    nc.scalar.activation(out=sq, in_=psk[mc4],
                                         func=AF.Square,
                                         accum_out=stats[:, 8 + m:9 + m])

        if _DEBUG_CUT == 3:           # end after phase F (no stats AllReduce)
            _cut(outpre[0:1, 0, :])
            return

        # single merged BN stats AllReduce (both heads)
        bn_in = dpool.tile([128, 16], F32, name="bn_in")
        bn_out = dpool.tile([128, 16], F32, name="bn_out",
                            addr_space="Shared")
        nc.sync.dma_start(out=bn_in, in_=stats)
        nc.gpsimd.collective_compute(
            "AllReduce", ALU.add,
            replica_groups=[list(range(NCORES))],
            ins=[bn_in.opt()], outs=[bn_out.opt()])
        g = acts.tile([128, 16], F32, tag="gstats", name="gstats")
        nc.sync.dma_start(out=g, in_=bn_out)

        if _DEBUG_CUT == 4:           # end after stats AllReduce
            z = tmp.tile([1, NL], F32, tag="cutz", name="cutz")
            nc.vector.memset(z, 0.0)
            nc.vector.tensor_copy(z[:, 0:16], g[0:1, :])
            nc.sync.dma_start(out=s_out[:], in_=z)
            return

        # ================= phase G: BN math + normalize into bn_all
        epsc = acts.tile([128, 1], F32, tag="epsc", name="epsc")
        nc.vector.memset(epsc, EPS_BN)
        mean = acts.tile([128, KD], F32, tag="bn_mean", name="bn_mean")
        nc.vector.tensor_scalar(out=mean, in0=g[:, 0:8],
                                scalar1=1.0 / NTOT, scalar2=None,
                                op0=ALU.mult)
        var = acts.tile([128, KD], F32, tag="bn_var", name="bn_var")
        nc.vector.tensor_scalar(out=var, in0=g[:, 8:16],
                                scalar1=1.0 / NTOT, scalar2=None,
                                op0=ALU.mult)
        msq = tmp.tile([128, KD], F32, tag="bn_msq", name="bn_msq")
        nc.vector.tensor_tensor(out=msq, in0=mean, in1=mean, op=ALU.mult)
        nc.vector.tensor_tensor(out=var, in0=var, in1=msq, op=ALU.subtract)
        # rstd = exp(-0.5*ln(var+eps)) stays inside the one ACT table set
        lnv = tmp.tile([128, KD], F32, tag="bn_lnv", name="bn_lnv")
        nc.scalar.activation(out=lnv, in_=var, func=AF.Ln, bias=epsc)
        rstd = tmp.tile([128, KD], F32, tag="bn_rstd", name="bn_rstd")
        nc.scalar.activation(out=rstd, in_=lnv, func=AF.Exp, scale=-0.5)
        sg = acts.tile([128, KD], F32, tag="bn_sg", name="bn_sg")
        nc.vector.tensor_tensor(out=sg, in0=cb['gamma_t'][:], in1=rstd,
                                op=ALU.mult)
        shift = acts.tile([128, KD], F32, tag="bn_shift", name="bn_shift")
        nc.vector.tensor_tensor(out=shift, in0=mean, in1=sg, op=ALU.mult)
        nc.vector.tensor_tensor(out=shift, in0=cb['beta_t'][:], in1=shift,
                                op=ALU.subtract)
        bn_all = acts.tile([128, KD, NL], MLP_DT, tag="bn_all", name="bn_all")
        for m in range(KD):
            move(m, bn_all[:, m, :], outpre[:, m, :],
                 bias=shift[:, m:m + 1], scale=sg[:, m:m + 1])

        if _DEBUG_CUT == 5:           # end after phase G
            _cut(outpre[0:1, 0, :])
            return

        # ================= phase H: MLP head
        with tc.tile_pool(name="ph", bufs=6, space="PSUM") as phl:
            # tT[s, m] = tgtcols^T @ W1b : psum [8, 512] x2
            tT = acts.tile([8, DIM], BF16, tag="tT", name="tT")
            tgt4 = bn_all.rearrange("p k (s t) -> p k s t", t=64)
            for half in range(2):
                ps = pmm.tile([8, DH], F32, tag="mm", name=f"ps_tT_{half}")
                hsl = slice(half * DH, (half + 1) * DH)
                if has_b1:
                    mm(ps, cb['ones_row'][:, :8], cb['b1r'][:, hsl], True)
                if W1_FP8:
                    for j in range(KD // 2):
                        jsl = slice(2 * j, 2 * j + 2)
                        mmdr(ps, tgt4[:, jsl, :, 63:64],
                             cb['w1b'][:, jsl, hsl],
                             (j == 0) and not has_b1, j == KD // 2 - 1)
                else:
                    for kc in range(KD):
                        mm(ps, tgt4[:, kc, :, 63:64], cb['w1b'][:, kc, hsl],
                           (kc == 0) and not has_b1, kc == KD - 1)
                nc.vector.tensor_copy(tT[:, hsl], ps)

            hsb = acts.tile([128, KD, NL], BF16, tag="hsb", name="hsb")
            for ms in ([0, 1, 2, 3, 4, 5], [6, 7]):
                ph = {m: phl.tile([128, NL], F32, tag="hpsum",
                                  name=f"ps_h_{m}") for m in ms}
                if W1_FP8:
                    for j in range(KD // 2):
                        jsl = slice(2 * j, 2 * j + 2)
                        for m in ms:
                            msl = slice(m * 128, (m + 1) * 128)
                            mmdr(ph[m], cb['w1a'][:, jsl, msl],
                                 bn_all[:, jsl, :], j == 0, False)
                else:
                    for kc in range(KD):
                        for m in ms:
                            msl = slice(m * 128, (m + 1) * 128)
                            mm(ph[m], cb['w1a'][:, kc, msl],
                               bn_all[:, kc, :], kc == 0, False)
                for m in ms:
                    msl = slice(m * 128, (m + 1) * 128)
                    mm(ph[m], tT[:, msl], cb['sel'], False, True)
                    # psum = S_BN*S_W1*h_pre ; hsb = S_H*relu(h_pre)
                    hs = 1.0 / (S_BN * S_W1) if W1_FP8 else None
                    move(m, hsb[:, m, :], ph[m], relu=True, scale=hs)

            pz = pmm.tile([1, NL], F32, tag="mm", name="ps_z")
            for m in range(KD):
                mm(pz, cb['w2'][:, m:m + 1], hsb[:, m, :],
                   m == 0, m == KD - 1)
            zs = -1.0
            esb = acts.tile([1, NL], F32, tag="esb", name="esb")
            if has_b2:
                nc.scalar.activation(out=esb, in_=pz, func=AF.Exp, scale=zs,
                                     bias=cb['negb2'][:, 0:1])
            else:
                nc.scalar.activation(out=esb, in_=pz, func=AF.Exp, scale=zs)
            nc.vector.tensor_scalar(out=esb, in0=esb, scalar1=1.0,
                                    scalar2=None, op0=ALU.add)
            ssb = acts.tile([1, NL], F32, tag="ssb", name="ssb")
            nc.vector.reciprocal_approx_fast(out=ssb, in_=esb)
            nc.sync.dma_start(out=s_out[:], in_=ssb)


# ------------------------------------------------------------------ driver
def kernel(_bass_trace=False, **inputs):
    global LAST_EXEC_NS, LAST_RESULTS
    per_core, flags = _host_prep(inputs)

    if flags not in _COMPILED:
        _COMPILED[flags] = _build_program(flags)
    nc = _COMPILED[flags]

    in_maps = []
    for c in range(NCORES):
        m = {}
        for name in _INPUT_SHAPES:
            m[name] = np.ascontiguousarray(per_core[c][name],
                                           dtype=_np_dt(name))
        in_maps.append(m)

    res = run_bass_kernel_spmd(nc, in_maps, list(range(NCORES)),
                               trace=_bass_trace)
    LAST_EXEC_NS = res.exec_time_ns
    LAST_RESULTS = res

    f = np.full((B, 512), -1.0, np.float32)
    for c in range(NCORES):
        f[c * BL:(c + 1) * BL, :S] = \
            np.asarray(res.results[c]['s_out'], np.float32).reshape(BL, S)
    mask = np.zeros((B, 512), np.int32)
    mask[:, :S] = 1
    return f, mask
